# revision 1
# baseline (speedup 1.0000x reference)
"""Trainium2 Bass kernel for the AttentionCritic problem.

Strategy (pure data-parallel over batch, 8 cores):
  - Host: transpose states/actions to feature-major, cast to bf16, pack
    per-head weights into merged [128,128] matrices, precompute the
    argmax one-hot (exactly matches reference tie-breaking), build the
    small selector/ones constant matrices used for PE-based partition
    reductions/broadcasts.
  - Device (per core, batch shard 4096, feature-major layout
    [feature_on_partitions, batch_on_free]):
      dense encoders / K,Q,V / critic on TensorE (bf16),
      relu/exp/drains on ScalarE,
      pairwise QK and attn*V products on VectorE (bf16 2x mode),
      cross-partition reductions (sum over d, sum over j) and
      partition broadcasts via ones/selector matmuls on TensorE.
"""

import sys

sys.path.insert(0, "/opt/trn_rl_repo")

import numpy as np
import ml_dtypes

N, B, SDIM, ADIM, HID, HEADS = 8, 32768, 128, 16, 128, 4
AD = HID // HEADS
IDIM = SDIM + ADIM
NCORES = 8
BSH = B // NCORES
BF16 = ml_dtypes.bfloat16
INV_SQRT_AD = 1.0 / np.sqrt(AD).astype(np.float32)


def build_nc(bsh, F=512, S=512, split=True, phases='dac'):
    """Build the Bass module for one core processing a batch shard of bsh."""
    import concourse.bass as bass
    import concourse.mybir as mybir
    from concourse.tile import TileContext

    f32 = mybir.dt.float32
    bf16 = mybir.dt.bfloat16
    MULT = mybir.AluOpType.mult
    RELU = mybir.ActivationFunctionType.Relu
    EXP = mybir.ActivationFunctionType.Exp
    IDENT = mybir.ActivationFunctionType.Identity
    COPY = mybir.ActivationFunctionType.Copy

    F = min(F, bsh)
    S = min(S, F)
    n_chunks = bsh // F
    n_sub = F // S

    nc = bass.Bass()

    # ---- DRAM parameters ----
    dp = nc.declare_dram_parameter
    sT = dp("sT", [N, SDIM, bsh], bf16, isOutput=False)
    aT = dp("aT", [N, ADIM, bsh], bf16, isOutput=False)
    we1 = dp("we1", [N, SDIM, HID], bf16, isOutput=False)
    we2 = dp("we2", [N, ADIM, HID], bf16, isOutput=False)
    ws = dp("ws", [N, SDIM, HID], bf16, isOutput=False)
    wk = dp("wk", [HID, HID], bf16, isOutput=False)
    wq = dp("wq", [HID, HID], bf16, isOutput=False)
    wv = dp("wv", [HID, HID], bf16, isOutput=False)
    wc1a = dp("wc1a", [N, HID, HID], bf16, isOutput=False)
    wc1b = dp("wc1b", [N, HID, HID], bf16, isOutput=False)
    wc2 = dp("wc2", [N, HID, ADIM], bf16, isOutput=False)
    ones_red = dp("ones_red", [HID, N * 32], bf16, isOutput=False)
    sel_bc = dp("sel_bc", [32, N * HID], bf16, isOutput=False)
    ident = dp("ident", [HID, HID], bf16, isOutput=False)
    sel_d = dp("sel_d", [N, 32, 4], bf16, isOutput=False)
    sel_rbc = dp("sel_rbc", [4, HID], f32, isOutput=False)
    ones16 = dp("ones16", [ADIM, 1], f32, isOutput=False)
    bE = dp("bE", [N, HID, 1], f32, isOutput=False)
    bS = dp("bS", [N, HID, 1], f32, isOutput=False)
    bV = dp("bV", [HID, 1], f32, isOutput=False)
    bC1 = dp("bC1", [N, HID, 1], f32, isOutput=False)
    bC2 = dp("bC2", [N, ADIM, 1], f32, isOutput=False)
    q8 = dp("q8", [N, ADIM, bsh], f32, isOutput=True)

    with TileContext(nc) as tc:
        # ---- constants (loaded once) ----
        with tc.tile_pool(name="const", bufs=1) as cp:

            def cload(ap, shape, dt, tag):
                t = cp.tile(shape, dt, tag=tag)
                nc.sync.dma_start(out=t[:], in_=ap)
                return t

            we1_s = [cload(we1[n], [SDIM, HID], bf16, f"we1{n}") for n in range(N)]
            we2_s = [cload(we2[n], [ADIM, HID], bf16, f"we2{n}") for n in range(N)]
            ws_s = [cload(ws[n], [SDIM, HID], bf16, f"ws{n}") for n in range(N)]
            wk_s = cload(wk[:], [HID, HID], bf16, "wk")
            wq_s = cload(wq[:], [HID, HID], bf16, "wq")
            wv_s = cload(wv[:], [HID, HID], bf16, "wv")
            wc1a_s = [cload(wc1a[n], [HID, HID], bf16, f"wc1a{n}") for n in range(N)]
            wc1b_s = [cload(wc1b[n], [HID, HID], bf16, f"wc1b{n}") for n in range(N)]
            wc2_s = [cload(wc2[n], [HID, ADIM], bf16, f"wc2{n}") for n in range(N)]
            onesr_s = cload(ones_red[:], [HID, N * 32], bf16, "onesr")
            selbc_s = cload(sel_bc[:], [32, N * HID], bf16, "selbc")
            ident_s = cload(ident[:], [HID, HID], bf16, "ident")
            seld_s = [cload(sel_d[n], [32, 4], bf16, f"seld{n}") for n in range(N)]
            selr_s = cload(sel_rbc[:], [4, HID], f32, "selr")
            ones16_s = cload(ones16[:], [ADIM, 1], f32, "ones16")
            bE_s = [cload(bE[n], [HID, 1], f32, f"bE{n}") for n in range(N)]
            bS_s = [cload(bS[n], [HID, 1], f32, f"bS{n}") for n in range(N)]
            bV_s = cload(bV[:], [HID, 1], f32, "bV")
            bC1_s = [cload(bC1[n], [HID, 1], f32, f"bC1{n}") for n in range(N)]
            bC2_s = [cload(bC2[n], [ADIM, 1], f32, f"bC2{n}") for n in range(N)]

            with (
                tc.tile_pool(name="persist", bufs=2) as pp,
                tc.tile_pool(name="work", bufs=3) as wp,
                tc.tile_pool(name="psum", bufs=2, space="PSUM") as qp,
            ):
                for c in range(n_chunks):
                    c0 = c * F
                    # persistent per-chunk tiles
                    senc = [pp.tile([HID, F], bf16, tag=f"senc{n}", name=f"senc{n}") for n in range(N)]
                    Kt = [pp.tile([HID, F], bf16, tag=f"K{n}", name=f"K{n}") for n in range(N)]
                    Qt = [pp.tile([HID, F], bf16, tag=f"Q{n}", name=f"Q{n}") for n in range(N)]
                    Vt = [pp.tile([HID, F], bf16, tag=f"V{n}", name=f"V{n}") for n in range(N)]
                    Ot = [pp.tile([HID, F], bf16, tag=f"O{n}", name=f"O{n}") for n in range(N)]

                    # ---------- dense phase ----------
                    for n in range(N):
                        st = wp.tile([SDIM, F], bf16, tag="st", name="st")
                        at = wp.tile([ADIM, F], bf16, tag="at", name="at")
                        nc.sync.dma_start(out=st[:], in_=sT[n, :, c0 : c0 + F])
                        nc.sync.dma_start(out=at[:], in_=aT[n, :, c0 : c0 + F])
                        saenc = wp.tile([HID, F], bf16, tag="saenc", name="saenc")
                        for s in range(n_sub):
                            sl = slice(s * S, (s + 1) * S)
                            ps = qp.tile([HID, S], f32, tag="ps", name="ps")
                            nc.tensor.matmul(ps[:], we1_s[n][:], st[:, sl], start=True, stop=False)
                            nc.tensor.matmul(ps[:], we2_s[n][:], at[:, sl], start=False, stop=True)
                            nc.scalar.activation(saenc[:, sl], ps[:], RELU, bias=bE_s[n][:])
                            ps2 = qp.tile([HID, S], f32, tag="ps", name="ps")
                            nc.tensor.matmul(ps2[:], ws_s[n][:], st[:, sl], start=True, stop=True)
                            nc.scalar.activation(senc[n][:, sl], ps2[:], RELU, bias=bS_s[n][:])
                            psk = qp.tile([HID, S], f32, tag="ps", name="ps")
                            nc.tensor.matmul(psk[:], wk_s[:], saenc[:, sl], start=True, stop=True)
                            nc.vector.tensor_copy(Kt[n][:, sl], psk[:])
                            psq = qp.tile([HID, S], f32, tag="ps", name="ps")
                            nc.tensor.matmul(psq[:], wq_s[:], senc[n][:, sl], start=True, stop=True)
                            nc.vector.tensor_copy(Qt[n][:, sl], psq[:])
                            psv = qp.tile([HID, S], f32, tag="ps", name="ps")
                            nc.tensor.matmul(psv[:], wv_s[:], saenc[:, sl], start=True, stop=True)
                            nc.scalar.activation(Vt[n][:, sl], psv[:], RELU, bias=bV_s[:])

                    # ---------- attention phase ----------
                    for i in range(N if 'a' in phases else 0):
                        Ei = wp.tile([32, F], bf16, tag="E", name="E")
                        Rbc = wp.tile([HID, F], f32, tag="Rbc", name="Rbc")
                        for s in range(n_sub):
                            sl = slice(s * S, (s + 1) * S)
                            # logits: L[(j,k), b] = sum_d Q_i[(k,d),b] K_j[(k,d),b]
                            Lp = qp.tile([32, S], f32, tag="L", name="L")
                            js = [j for j in range(N) if j != i]
                            for idx, j in enumerate(js):
                                pr = wp.tile([HID, S], bf16, tag="prod", name="prod")
                                nc.vector.tensor_tensor(pr[:], Qt[i][:, sl], Kt[j][:, sl], MULT)
                                nc.tensor.matmul(
                                    Lp[:],
                                    onesr_s[:, j * 32 : (j + 1) * 32],
                                    pr[:],
                                    start=(idx == 0),
                                    stop=(idx == len(js) - 1),
                                )
                            nc.scalar.activation(Ei[:, sl], Lp[:], EXP, scale=float(INV_SQRT_AD))
                            # denominator and its reciprocal, broadcast to (k,d)
                            Dp = qp.tile([4, S], f32, tag="D", name="D", bufs=1)
                            nc.tensor.matmul(Dp[:], seld_s[i][:], Ei[:, sl], start=True, stop=True)
                            Rs = wp.tile([4, S], f32, tag="R", name="R")
                            nc.vector.reciprocal(Rs[:], Dp[:])
                            Rp = qp.tile([HID, S], f32, tag="bk", name="bk")
                            nc.tensor.matmul(Rp[:], selr_s[:], Rs[:], start=True, stop=True)
                            nc.scalar.activation(Rbc[:, sl], Rp[:], COPY)
                            # numerator: On[(k,d), b] = sum_j E[(j,k), b] * V_j[(k,d), b]
                            On = qp.tile([HID, S], f32, tag="On", name="On", bufs=1)
                            for idx, j in enumerate(js):
                                Ebp = qp.tile([HID, S], f32, tag="bk", name="bk")
                                nc.tensor.matmul(
                                    Ebp[:],
                                    selbc_s[:, j * HID : (j + 1) * HID],
                                    Ei[:, sl],
                                    start=True,
                                    stop=True,
                                )
                                Ebc = wp.tile([HID, S], bf16, tag="Ebc", name="Ebc")
                                nc.scalar.activation(Ebc[:], Ebp[:], COPY)
                                pr2 = wp.tile([HID, S], bf16, tag="prod2", name="prod2")
                                nc.vector.tensor_tensor(pr2[:], Ebc[:], Vt[j][:, sl], MULT)
                                nc.tensor.matmul(
                                    On[:],
                                    ident_s[:],
                                    pr2[:],
                                    start=(idx == 0),
                                    stop=(idx == len(js) - 1),
                                )
                            nc.vector.tensor_tensor(Ot[i][:, sl], On[:], Rbc[:, sl], MULT)

                    # ---------- critic phase ----------
                    if 'c' not in phases:
                        for n in range(N):
                            qs0 = wp.tile([ADIM, F], f32, tag="qs", name="qs")
                            nc.scalar.activation(qs0[:], (Kt[n][:ADIM, :] if 'a' not in phases else Ot[n][:ADIM, :]), COPY)
                            nc.sync.dma_start(out=q8[n, :, c0 : c0 + F], in_=qs0[:])
                    for n in range(N if 'c' in phases else 0):
                        for s in range(n_sub):
                            sl = slice(s * S, (s + 1) * S)
                            h1p = qp.tile([HID, S], f32, tag="ps", name="ps")
                            nc.tensor.matmul(h1p[:], wc1a_s[n][:], senc[n][:, sl], start=True, stop=False)
                            nc.tensor.matmul(h1p[:], wc1b_s[n][:], Ot[n][:, sl], start=False, stop=True)
                            h1 = wp.tile([HID, S], bf16, tag="h1", name="h1")
                            nc.scalar.activation(h1[:], h1p[:], RELU, bias=bC1_s[n][:])
                            aqp = qp.tile([ADIM, S], f32, tag="D", name="D", bufs=1)
                            nc.tensor.matmul(aqp[:], wc2_s[n][:], h1[:], start=True, stop=True)
                            aq = wp.tile([ADIM, S], f32, tag="aq", name="aq")
                            nc.scalar.activation(aq[:], aqp[:], IDENT, bias=bC2_s[n][:])
                            nc.sync.dma_start(out=q8[n, :, c0 + s * S : c0 + (s + 1) * S], in_=aq[:])
    if split:
        split_multi_waits(nc)
    return nc


def split_multi_waits(nc):
    """The 64B ISA instruction structs carry exactly ONE sync-wait slot.
    Tile emits instructions with several waits; walrus rejects them
    ("Too many sync wait commands").  Hoist all but one wait of each
    instruction onto a chain of same-engine NoOps placed directly before
    it in the instruction stream (queue-level stall, no pipe flush)."""
    import concourse.mybir as mybir

    nid = [0]
    for f in nc.m.functions:
        for blk in f.blocks:
            il = blk.instructions
            i = 0
            while i < len(il):
                inst = il[i]
                si = inst.sync_info
                if si is not None and si.on_wait and len(si.on_wait) > 1:
                    waits = list(si.on_wait)
                    extra, keep = waits[:-1], waits[-1:]
                    si.on_wait = keep
                    for w in extra:
                        nid[0] += 1
                        nop = mybir.InstNoOp(name=f"W-split-{nid[0]}", ins=[], outs=[])
                        nop.engine = inst.engine
                        nop.sync_info = mybir.SyncInfo(on_wait=[w], on_update=[])
                        il.insert(i, nop)
                        i += 1
                i += 1
    return nc


def host_prep(states, actions, We, be, Ws, bs, Wk, Wq, Wv, bv, Wc1, bc1, Wc2, bc2):
    """Pack/cast all tensors host-side. Returns (shared_inputs, per_core_fn)."""
    f32 = np.float32

    def bf(x):
        return np.ascontiguousarray(x, dtype=BF16)

    acs = np.argmax(actions, axis=-1)  # [N, B] (matches reference tie-breaking)

    # merged head weights: [h, k*AD+d]
    wk_m = np.concatenate([Wk[k] for k in range(HEADS)], axis=1)
    wq_m = np.concatenate([Wq[k] for k in range(HEADS)], axis=1)
    wv_m = np.concatenate([Wv[k] for k in range(HEADS)], axis=1)
    bv_m = np.concatenate([bv[k] for k in range(HEADS)], axis=0)  # [128]

    ones_red = np.zeros((HID, N * 32), f32)
    for j in range(N):
        for k in range(HEADS):
            ones_red[32 * k : 32 * (k + 1), 32 * j + 4 * j + k] = 1.0
    sel_bc = np.zeros((32, N * HID), f32)
    for j in range(N):
        for k in range(HEADS):
            sel_bc[4 * j + k, HID * j + 32 * k : HID * j + 32 * (k + 1)] = 1.0
    sel_d = np.zeros((N, 32, 4), f32)
    for i in range(N):
        for j in range(N):
            if j != i:
                for k in range(HEADS):
                    sel_d[i, 4 * j + k, k] = 1.0
    sel_rbc = np.zeros((4, HID), f32)
    for k in range(HEADS):
        sel_rbc[k, 32 * k : 32 * (k + 1)] = 1.0

    shared = {
        "we1": bf(We[:, :SDIM, :]),
        "we2": bf(We[:, SDIM:, :]),
        "ws": bf(Ws),
        "wk": bf(wk_m),
        "wq": bf(wq_m),
        "wv": bf(wv_m),
        "wc1a": bf(Wc1[:, :HID, :]),
        "wc1b": bf(Wc1[:, HID:, :]),
        "wc2": bf(Wc2),
        "ones_red": bf(ones_red),
        "sel_bc": bf(sel_bc),
        "ident": bf(np.eye(HID, dtype=f32)),
        "sel_d": bf(sel_d),
        "sel_rbc": np.ascontiguousarray(sel_rbc),
        "ones16": np.ones((ADIM, 1), f32),
        "bE": np.ascontiguousarray(be[..., None], f32),
        "bS": np.ascontiguousarray(bs[..., None], f32),
        "bV": np.ascontiguousarray(bv_m[..., None], f32),
        "bC1": np.ascontiguousarray(bc1[..., None], f32),
        "bC2": np.ascontiguousarray(bc2[..., None], f32),
    }
    sT_full = bf(states.transpose(0, 2, 1))  # [N, 128, B]
    aT_full = bf(actions.transpose(0, 2, 1))  # [N, 16, B]

    def core_inputs(c, bsh):
        lo = c * bsh
        return dict(
            shared,
            sT=np.ascontiguousarray(sT_full[:, :, lo : lo + bsh]),
            aT=np.ascontiguousarray(aT_full[:, :, lo : lo + bsh]),
        )

    return core_inputs, acs


def kernel(**inputs):
    from concourse.bass_utils import run_bass_kernel_spmd

    nc = build_nc(BSH)
    core_inputs, acs = host_prep(**inputs)
    in_maps = [core_inputs(c, BSH) for c in range(NCORES)]
    res = run_bass_kernel_spmd(nc, in_maps, list(range(NCORES))).results
    out = np.empty((N, B, 1), np.float32)
    for c in range(NCORES):
        aq = res[c]["q8"]  # [N, 16, BSH]
        sl = slice(c * BSH, (c + 1) * BSH)
        out[:, sl, 0] = np.take_along_axis(aq, acs[:, None, sl], axis=1)[:, 0, :]
    return out

